# revision 1
# baseline (speedup 1.0000x reference)
"""Trainium2 Bass kernel for a 4-layer residual LSTM encoder (Keras-style).

Problem: x [64, 512, 512] fp32, per-layer kernels W/U [4, 512, 2048], bias
[4, 2048]; returns (rnn_output [64,512,512], states [4,64,512]) matching the
reference (states = layer-0 final hidden state replicated 4x).

Sharding: data-parallel, batch 64 -> 8 NeuronCores x 8. Each core runs the
full 4-layer LSTM on its batch shard. Per layer, a "phase A" matmul projects
the whole sequence (zx = x@W + b, fp16, gate-grouped layout in DRAM), then
"phase B" walks the T=512 recurrence with U fp16-stationary on the PE
(fast-weight-load) streaming hT, with gates/cell updates on ACT/DVE in a
packed [128, (k, b)] layout so the new h directly becomes the next matmul's
rhs (hidden on partitions, no transposes).
"""

from contextlib import ExitStack

import numpy as np
import ml_dtypes

import concourse.bass as bass
import concourse.mybir as mybir
import concourse.tile as tile
from concourse import bacc
from concourse import bass_utils

FP16 = mybir.dt.float16
FP32 = mybir.dt.float32
AF = mybir.ActivationFunctionType
ds = bass.ds
HINTS = tuple(mybir.ALL_ENGINES)

B, T, H, L = 64, 512, 512, 4
NCORES = 8
Bl = B // NCORES


def _build_lstm(nc, L=4, T=512, Bl=8, H=512, C=16):
    KT = H // 128            # 4 hidden chunks
    G4 = 4 * H               # 2048 gate columns
    JT = G4 // 128           # 16 j-tiles
    TB = T * Bl              # activation free dim
    GW = KT * Bl             # packed gate-tile width (j4, b)
    CB = min(512, TB)        # phase-A column block
    NBLK = TB // CB
    Tpb = CB // Bl           # timesteps per phase-A block
    assert T % C == 0 and TB % CB == 0

    xT = nc.dram_tensor("xT", [H, TB], FP16, kind="ExternalInput").ap()
    W = nc.dram_tensor("W", [L, KT, 128, G4], FP16, kind="ExternalInput").ap()
    U = nc.dram_tensor("U", [L, KT, 128, G4], FP16, kind="ExternalInput").ap()
    bias = nc.dram_tensor("bias", [L, 128, JT], FP32, kind="ExternalInput").ap()
    yT = nc.dram_tensor("yT", [H, TB], FP16, kind="ExternalOutput").ap()
    h0f = nc.dram_tensor("h0f", [H, Bl], FP16, kind="ExternalOutput").ap()

    with tile.TileContext(nc) as tc:
        with ExitStack() as top:
            dram = top.enter_context(tc.tile_pool(name="dram", bufs=1, space="DRAM"))
            actA = dram.tile([H, TB], FP16)
            actB = dram.tile([H, TB], FP16)
            zx = dram.tile([4, 128, T, KT, Bl], FP16)
            state = top.enter_context(tc.tile_pool(name="state", bufs=1))
            h_st = state.tile([128, GW], FP16)
            c_st = state.tile([128, GW], FP32)

            for l in range(L):
                inp = xT if l == 0 else (actA if l % 2 == 1 else actB)
                outb = yT if l == L - 1 else (actA if l % 2 == 0 else actB)
                with ExitStack() as ls:
                    wp = ls.enter_context(tc.tile_pool(name=f"wp{l}", bufs=1))
                    U_sb = []
                    W_sb = []
                    for k in range(KT):
                        u_t = wp.tile([128, G4], FP16, name=f"u{l}_{k}")
                        nc.sync.dma_start(u_t[:], U[l, k])
                        U_sb.append(u_t)
                        w_t = wp.tile([128, G4], FP16, name=f"w{l}_{k}")
                        nc.sync.dma_start(w_t[:], W[l, k])
                        W_sb.append(w_t)
                    b_sb = wp.tile([128, JT], FP32, name=f"b{l}")
                    nc.sync.dma_start(b_sb[:], bias[l])

                    # ---- phase A: zx = inp @ W + b ----
                    with tc.tile_pool(name=f"pain{l}", bufs=1) as pain, \
                         tc.tile_pool(name=f"paps{l}", bufs=2, space="PSUM") as paps, \
                         tc.tile_pool(name=f"pao{l}", bufs=3) as pao:
                        in_sb = []
                        for k in range(KT):
                            it = pain.tile([128, TB], FP16, name=f"pain{l}_{k}")
                            nc.sync.dma_start(it[:], inp[k * 128:(k + 1) * 128, :])
                            in_sb.append(it)
                        for g in range(4):
                            for j4 in range(KT):
                                jt = g * KT + j4
                                for nb in range(NBLK):
                                    ps = paps.tile([128, CB], FP32, tag="pa_ps")
                                    for k in range(KT):
                                        nc.tensor.matmul(
                                            ps[:],
                                            lhsT=W_sb[k][:, jt * 128:(jt + 1) * 128],
                                            rhs=in_sb[k][:, nb * CB:(nb + 1) * CB],
                                            start=(k == 0), stop=(k == KT - 1),
                                        )
                                    zxs = pao.tile([128, CB], FP16, tag="pa_out")
                                    nc.vector.tensor_scalar_add(
                                        zxs[:], ps[:], b_sb[:, jt:jt + 1])
                                    nc.sync.dma_start(
                                        zx[g, :, nb * Tpb:(nb + 1) * Tpb, j4, :],
                                        zxs[:].rearrange("p (t b) -> p t b", b=Bl),
                                    )

                    # ---- phase B: recurrence ----
                    nc.vector.memset(h_st[:], 0.0)
                    nc.vector.memset(c_st[:], 0.0)
                    with tc.tile_pool(name=f"pbio{l}", bufs=2) as pbio, \
                         tc.tile_pool(name=f"pbps{l}", bufs=2, space="PSUM") as pbps, \
                         tc.tile_pool(name=f"pbg{l}", bufs=2) as pbg:
                        with tc.For_i(0, T, C, hint_engines=HINTS,
                                      name=f"rec{l}") as tv:
                            zxc = []
                            for g in range(4):
                                zt = pbio.tile([128, C * GW], FP16,
                                               tag=f"zxc{g}", name=f"zxc{l}_{g}")
                                nc.sync.dma_start(
                                    zt[:].rearrange("p (t j b) -> p t j b",
                                                    j=KT, b=Bl),
                                    zx[g, :, ds(tv, C), :, :],
                                )
                                zxc.append(zt)
                            inc = []
                            if l > 0:
                                for k in range(KT):
                                    it = pbio.tile([128, C * Bl], FP16,
                                                   tag=f"inc{k}", name=f"inc{l}_{k}")
                                    nc.sync.dma_start(
                                        it[:],
                                        inp[k * 128:(k + 1) * 128,
                                            ds(tv * Bl, C * Bl)],
                                    )
                                    inc.append(it)
                            outc = [pbio.tile([128, C * Bl], FP16, tag=f"outc{k}",
                                              name=f"outc{l}_{k}")
                                    for k in range(KT)]

                            for t in range(C):
                                ps_g = [pbps.tile([128, GW], FP32, tag=f"psg{g}",
                                                  name=f"ps{l}_{g}")
                                        for g in range(4)]
                                for g in range(4):
                                    for j4 in range(KT):
                                        jt = g * KT + j4
                                        for k in range(KT):
                                            nc.tensor.matmul(
                                                ps_g[g][:, j4 * Bl:(j4 + 1) * Bl],
                                                lhsT=U_sb[k][:, jt * 128:(jt + 1) * 128],
                                                rhs=h_st[:, k * Bl:(k + 1) * Bl],
                                                start=(k == 0), stop=(k == KT - 1),
                                            )
                                acts = [AF.Sigmoid, AF.Sigmoid, AF.Tanh, AF.Sigmoid]
                                gt = []
                                for g in range(4):
                                    zt = pbg.tile([128, GW], FP32, tag=f"z{g}",
                                                  name=f"zt{l}_{g}")
                                    nc.vector.tensor_add(
                                        zt[:], ps_g[g][:],
                                        zxc[g][:, t * GW:(t + 1) * GW])
                                    at = pbg.tile([128, GW], FP32, tag=f"g{g}",
                                                  name=f"gt{l}_{g}")
                                    nc.scalar.activation(at[:], zt[:], acts[g])
                                    gt.append(at)
                                sig_i, sig_f, tanh_g, sig_o = gt
                                t1 = pbg.tile([128, GW], FP32, tag="t1", name=f"t1{l}")
                                nc.vector.tensor_mul(t1[:], sig_f[:], c_st[:])
                                t2 = pbg.tile([128, GW], FP32, tag="t2", name=f"t2{l}")
                                nc.vector.tensor_mul(t2[:], sig_i[:], tanh_g[:])
                                nc.vector.tensor_add(c_st[:], t1[:], t2[:])
                                tc_t = pbg.tile([128, GW], FP32, tag="tc", name=f"tc{l}")
                                nc.scalar.activation(tc_t[:], c_st[:], AF.Tanh)
                                nc.vector.tensor_mul(h_st[:], sig_o[:], tc_t[:])
                                for k in range(KT):
                                    if l > 0:
                                        nc.vector.tensor_add(
                                            outc[k][:, t * Bl:(t + 1) * Bl],
                                            h_st[:, k * Bl:(k + 1) * Bl],
                                            inc[k][:, t * Bl:(t + 1) * Bl])
                                    else:
                                        nc.vector.tensor_copy(
                                            outc[k][:, t * Bl:(t + 1) * Bl],
                                            h_st[:, k * Bl:(k + 1) * Bl])
                            for k in range(KT):
                                nc.sync.dma_start(
                                    outb[k * 128:(k + 1) * 128,
                                         ds(tv * Bl, C * Bl)],
                                    outc[k][:])
                            if l == 0:
                                for k in range(KT):
                                    nc.sync.dma_start(
                                        h0f[k * 128:(k + 1) * 128, :],
                                        h_st[:, k * Bl:(k + 1) * Bl])
    return nc


_CACHE = {}


def _get_compiled():
    if "nc" not in _CACHE:
        nc = bacc.Bacc("TRN2", target_bir_lowering=False, debug=False,
                       num_devices=NCORES)
        _build_lstm(nc, L=L, T=T, Bl=Bl, H=H, C=16)
        nc.compile()
        _CACHE["nc"] = nc
    return _CACHE["nc"]


def _make_in_maps(x, W, U, b):
    KT = H // 128
    W4 = np.ascontiguousarray(
        np.asarray(W, np.float32).reshape(L, KT, 128, 4 * H)).astype(np.float16)
    U4 = np.ascontiguousarray(
        np.asarray(U, np.float32).reshape(L, KT, 128, 4 * H)).astype(np.float16)
    b4 = np.ascontiguousarray(
        np.asarray(b, np.float32).reshape(L, 16, 128).transpose(0, 2, 1))
    x = np.asarray(x, np.float32)
    in_maps = []
    for ci in range(NCORES):
        xs = x[ci * Bl:(ci + 1) * Bl]                       # [Bl, T, H]
        xTl = np.ascontiguousarray(
            xs.transpose(2, 1, 0).reshape(H, T * Bl)).astype(np.float16)
        in_maps.append({"xT": xTl, "W": W4, "U": U4, "bias": b4})
    return in_maps


def _unpack(results):
    y = np.empty((B, T, H), np.float32)
    h0 = np.empty((B, H), np.float32)
    for ci in range(NCORES):
        yT = np.asarray(results[ci]["yT"], np.float32)       # [H, T*Bl]
        y[ci * Bl:(ci + 1) * Bl] = yT.reshape(H, T, Bl).transpose(2, 1, 0)
        h0[ci * Bl:(ci + 1) * Bl] = np.asarray(
            results[ci]["h0f"], np.float32).T                # [Bl, H]
    states = np.broadcast_to(h0, (L, B, H)).copy()
    return y, states


def _run(in_maps, **kw):
    nc = _get_compiled()
    return bass_utils.run_bass_kernel_spmd(
        nc, in_maps, core_ids=list(range(NCORES)), **kw)


def kernel(x, W, U, b):
    res = _run(_make_in_maps(x, W, U, b))
    return _unpack(res.results)


# revision 6
# speedup vs baseline: 1.9938x; 1.9938x over previous
"""Trainium2 Bass kernel for a 4-layer residual LSTM encoder (Keras-style).

Problem: x [64, 512, 512] fp32, per-layer kernels W/U [4, 512, 2048], bias
[4, 2048]; returns (rnn_output [64,512,512], states [4,64,512]) matching the
reference (states = layer-0 final hidden state replicated 4x).

Sharding: data-parallel, batch 64 -> 8 NeuronCores x 8. Each core runs the
full 4-layer LSTM on its batch shard.

Per layer the sequence is processed in segments of C timesteps with a
two-segment software pipeline inside an unrolled-2 For_i: while the
recurrence walks segment s, the input projection (zx = x@W + b) for segment
s+1 is computed by matmuls interleaved into the step stream (2 per step) and
kept entirely in SBUF. The recurrence streams hT through the PE with U
fp16-stationary (fast-weight-load eligible); gates are evaluated in a packed
[128, (gate, k, b)] layout on ACT/DVE so the new h lands directly in the
layout the next step's matmul needs (hidden on partitions, no transposes and
no DRAM roundtrips inside a layer).
"""

from contextlib import ExitStack

import numpy as np

import concourse.bass as bass
import concourse.mybir as mybir
import concourse.tile as tile
from concourse import bacc
from concourse import bass_utils

FP16 = mybir.dt.float16
FP32 = mybir.dt.float32
AF = mybir.ActivationFunctionType
ds = bass.ds
HINTS = tuple(mybir.ALL_ENGINES)

B, T, H, L = 64, 512, 512, 4
NCORES = 8
Bl = B // NCORES
C = 32                      # timesteps per segment
PAD = C * Bl                # input-column padding for the lookahead loads


def _build_lstm(nc, L=L, T=T, Bl=Bl, H=H, C=C):
    KT = H // 128            # 4 hidden chunks
    G4 = 4 * H               # 2048 gate columns
    TB = T * Bl              # activation free dim
    GW = KT * Bl             # 32: per-gate packed width (j4, b)
    PW = 2 * GW              # 64: gate-pair packed width (gi, j4, b)
    SW = C * Bl              # segment width in activation cols
    pad = C * Bl
    assert T % (2 * C) == 0

    xT = nc.dram_tensor("xT", [H, TB + pad], FP16, kind="ExternalInput").ap()
    W = nc.dram_tensor("W", [L, KT, 128, G4], FP16, kind="ExternalInput").ap()
    U = nc.dram_tensor("U", [L, KT, 128, G4], FP16, kind="ExternalInput").ap()
    bias = nc.dram_tensor("bias", [L, 128, 16], FP32, kind="ExternalInput").ap()
    yT = nc.dram_tensor("yT", [H, TB], FP16, kind="ExternalOutput").ap()
    h0f = nc.dram_tensor("h0f", [H, Bl], FP16, kind="ExternalOutput").ap()

    with tile.TileContext(nc) as tc:
        with ExitStack() as top:
            dram = top.enter_context(tc.tile_pool(name="dram", bufs=1, space="DRAM"))
            actA = dram.tile([H, TB + pad], FP16)
            actB = dram.tile([H, TB + pad], FP16)
            state = top.enter_context(tc.tile_pool(name="state", bufs=1))
            h_st = state.tile([128, GW], FP16)
            c_st = state.tile([128, GW], FP32)
            # zero the lookahead padding of the inter-layer buffers
            zpad = state.tile([128, pad], FP16)
            nc.vector.memset(zpad[:], 0.0)
            for buf in (actA, actB):
                for k in range(KT):
                    nc.sync.dma_start(
                        buf[k * 128:(k + 1) * 128, TB:TB + pad], zpad[:])

            for l in range(L):
                inp = xT if l == 0 else (actA if l % 2 == 1 else actB)
                outb = yT if l == L - 1 else (actA if l % 2 == 0 else actB)
                with ExitStack() as ls:
                    wp = ls.enter_context(tc.tile_pool(name=f"wp{l}", bufs=1))
                    U_sb, W_sb = [], []
                    for k in range(KT):
                        u_t = wp.tile([128, G4], FP16, name=f"u{l}_{k}")
                        nc.sync.dma_start(u_t[:], U[l, k])
                        U_sb.append(u_t)
                        w_t = wp.tile([128, G4], FP16, name=f"w{l}_{k}")
                        nc.sync.dma_start(w_t[:], W[l, k])
                        W_sb.append(w_t)
                    b_sb = wp.tile([128, 16], FP32, name=f"b{l}")
                    nc.sync.dma_start(b_sb[:], bias[l])

                    # persistent ping-pong buffers for the segment pipeline
                    pp = ls.enter_context(tc.tile_pool(name=f"pp{l}", bufs=1))
                    # in_pp[p][k]: input chunk (k-tile) [128, SW]
                    in_pp = [[pp.tile([128, SW], FP16, name=f"in{l}_{p}_{k}")
                              for k in range(KT)] for p in range(2)]
                    # zx_pp[p][pair]: projected input segment [128, C*PW]
                    # free layout (t, gi, j4, b); pair 0 = gates (i,f), 1 = (g,o)
                    zx_pp = [[pp.tile([128, C * PW], FP16, name=f"zx{l}_{p}_{d}")
                              for d in range(2)] for p in range(2)]

                    paps = ls.enter_context(
                        tc.tile_pool(name=f"paps{l}", bufs=2, space="PSUM"))
                    pbps = ls.enter_context(
                        tc.tile_pool(name=f"pbps{l}", bufs=2, space="PSUM"))
                    pbg = ls.enter_context(tc.tile_pool(name=f"pbg{l}", bufs=2))
                    pout = ls.enter_context(tc.tile_pool(name=f"pout{l}", bufs=2))

                    def in_chunk_dma(dst_tiles, col0):
                        for k in range(KT):
                            nc.sync.dma_start(
                                dst_tiles[k][:],
                                inp[k * 128:(k + 1) * 128, ds(col0, SW)])

                    def proj_mms(in_tiles, zx_tiles, mlist):
                        """Emit phase-A matmuls (index list) for one segment.

                        MM m: jt = m//KT, k = m%KT; after the last k, evacuate
                        psum (+bias, cast fp16) into zx_tiles with a strided
                        DVE write at (t, gi, j4, b) position j4 of pair/gi.
                        """
                        for m in mlist:
                            jt, k = m // KT, m % KT
                            if k == 0:
                                proj_mms.cur[jt] = paps.tile(
                                    [128, SW], FP32, tag="pa_ps", name=f"pa{l}")
                            nc.tensor.matmul(
                                proj_mms.cur[jt][:],
                                lhsT=W_sb[k][:, jt * 128:(jt + 1) * 128],
                                rhs=in_tiles[k][:],
                                start=(k == 0), stop=(k == KT - 1),
                            )
                            if k == KT - 1:
                                pair, gi, j4 = jt // 8, (jt % 8) // KT, jt % KT
                                dst = zx_tiles[pair][:].rearrange(
                                    "p (t gi j b) -> p t gi j b",
                                    gi=2, j=KT, b=Bl)[:, :, gi, j4, :]
                                nc.vector.tensor_scalar_add(
                                    dst,
                                    proj_mms.cur[jt][:].rearrange(
                                        "p (t b) -> p t b", b=Bl),
                                    b_sb[:, jt:jt + 1])
                    proj_mms.cur = [None] * 16

                    def segment(p, col0, zx_in_p, write_h0f, emit_proj,
                                in_res, in_next, zx_out_p):
                        """One segment of C recurrence steps (+ interleaved
                        projection matmuls for the next segment)."""
                        zxr = zx_pp[zx_in_p]
                        outc = [pout.tile([128, SW], FP16, tag=f"outc{k}",
                                          name=f"outc{l}_{k}")
                                for k in range(KT)]
                        nproj = (64 + C - 1) // C
                        for t in range(C):
                            ps_pair = [pbps.tile([128, PW], FP32, tag=f"psp{d}",
                                                 name=f"psp{l}_{d}")
                                       for d in range(2)]
                            for d in range(2):
                                for gi in range(2):
                                    for j4 in range(KT):
                                        jt = (d * 2 + gi) * KT + j4
                                        sl = slice((gi * KT + j4) * Bl,
                                                   (gi * KT + j4 + 1) * Bl)
                                        for k in range(KT):
                                            nc.tensor.matmul(
                                                ps_pair[d][:, sl],
                                                lhsT=U_sb[k][:, jt * 128:(jt + 1) * 128],
                                                rhs=h_st[:, k * Bl:(k + 1) * Bl],
                                                start=(k == 0), stop=(k == KT - 1),
                                            )
                            if emit_proj:
                                proj_mms(in_next, zx_pp[zx_out_p],
                                         range(t * nproj, min(64, (t + 1) * nproj)))
                            # z = psum + zx segment slice; gates
                            zp = []
                            for d in range(2):
                                zt = pbg.tile([128, PW], FP32, tag=f"z{d}",
                                              name=f"zt{l}_{d}")
                                nc.vector.tensor_add(
                                    zt[:], ps_pair[d][:],
                                    zxr[d][:, t * PW:(t + 1) * PW])
                                zp.append(zt)
                            g_if = pbg.tile([128, PW], FP32, tag="gif",
                                            name=f"gif{l}")
                            nc.scalar.activation(g_if[:], zp[0][:], AF.Sigmoid)
                            g_go = pbg.tile([128, PW], FP32, tag="ggo",
                                            name=f"ggo{l}")
                            nc.scalar.activation(g_go[:, :GW], zp[1][:, :GW],
                                                 AF.Tanh)
                            nc.scalar.activation(g_go[:, GW:], zp[1][:, GW:],
                                                 AF.Sigmoid)
                            t1 = pbg.tile([128, GW], FP32, tag="t1", name=f"t1{l}")
                            nc.vector.tensor_mul(t1[:], g_if[:, GW:], c_st[:])
                            t2 = pbg.tile([128, GW], FP32, tag="t2", name=f"t2{l}")
                            nc.vector.tensor_mul(t2[:], g_if[:, :GW], g_go[:, :GW])
                            nc.vector.tensor_add(c_st[:], t1[:], t2[:])
                            tc_t = pbg.tile([128, GW], FP32, tag="tc", name=f"tc{l}")
                            nc.scalar.activation(tc_t[:], c_st[:], AF.Tanh)
                            nc.vector.tensor_mul(h_st[:], g_go[:, GW:], tc_t[:])
                            for k in range(KT):
                                if l > 0:
                                    nc.vector.tensor_add(
                                        outc[k][:, t * Bl:(t + 1) * Bl],
                                        h_st[:, k * Bl:(k + 1) * Bl],
                                        in_res[k][:, t * Bl:(t + 1) * Bl])
                                else:
                                    nc.vector.tensor_copy(
                                        outc[k][:, t * Bl:(t + 1) * Bl],
                                        h_st[:, k * Bl:(k + 1) * Bl])
                        for k in range(KT):
                            nc.sync.dma_start(
                                outb[k * 128:(k + 1) * 128, ds(col0, SW)],
                                outc[k][:])
                        if write_h0f:
                            for k in range(KT):
                                nc.sync.dma_start(
                                    h0f[k * 128:(k + 1) * 128, :],
                                    h_st[:, k * Bl:(k + 1) * Bl])

                    # ---- prologue: chunk 0 + its projection; zero state ----
                    in_chunk_dma(in_pp[0], 0)
                    proj_mms(in_pp[0], zx_pp[0], range(64))
                    nc.vector.memset(h_st[:], 0.0)
                    nc.vector.memset(c_st[:], 0.0)

                    with tc.For_i(0, T, 2 * C, hint_engines=HINTS,
                                  name=f"rec{l}") as tv:
                        # body 0: segment at cols tv*Bl, using zx0/in0;
                        # loads chunk(s+1) -> in1, projects -> zx1
                        in_chunk_dma(in_pp[1], tv * Bl + SW)
                        segment(0, tv * Bl, 0, l == 0, True,
                                in_pp[0], in_pp[1], 1)
                        # body 1: segment at cols (tv+C)*Bl using zx1/in1;
                        # loads chunk(s+2) -> in0, projects -> zx0
                        in_chunk_dma(in_pp[0], tv * Bl + 2 * SW)
                        segment(1, tv * Bl + SW, 1, l == 0, True,
                                in_pp[1], in_pp[0], 0)
    return nc


_CACHE = {}


def _get_compiled():
    if "nc" not in _CACHE:
        nc = bacc.Bacc("TRN2", target_bir_lowering=False, debug=False,
                       num_devices=NCORES)
        _build_lstm(nc)
        nc.compile()
        _CACHE["nc"] = nc
    return _CACHE["nc"]


def _make_in_maps(x, W, U, b):
    KT = H // 128
    W4 = np.ascontiguousarray(
        np.asarray(W, np.float32).reshape(L, KT, 128, 4 * H)).astype(np.float16)
    U4 = np.ascontiguousarray(
        np.asarray(U, np.float32).reshape(L, KT, 128, 4 * H)).astype(np.float16)
    b4 = np.ascontiguousarray(
        np.asarray(b, np.float32).reshape(L, 16, 128).transpose(0, 2, 1))
    x = np.asarray(x, np.float32)
    in_maps = []
    for ci in range(NCORES):
        xs = x[ci * Bl:(ci + 1) * Bl]                       # [Bl, T, H]
        xTl = np.zeros((H, T * Bl + PAD), np.float16)
        xTl[:, :T * Bl] = xs.transpose(2, 1, 0).reshape(H, T * Bl)
        in_maps.append({"xT": xTl, "W": W4, "U": U4, "bias": b4})
    return in_maps


def _unpack(results):
    y = np.empty((B, T, H), np.float32)
    h0 = np.empty((B, H), np.float32)
    for ci in range(NCORES):
        yT = np.asarray(results[ci]["yT"], np.float32)       # [H, T*Bl]
        y[ci * Bl:(ci + 1) * Bl] = yT.reshape(H, T, Bl).transpose(2, 1, 0)
        h0[ci * Bl:(ci + 1) * Bl] = np.asarray(
            results[ci]["h0f"], np.float32).T                # [Bl, H]
    states = np.broadcast_to(h0, (L, B, H)).copy()
    return y, states


def _run(in_maps, **kw):
    nc = _get_compiled()
    return bass_utils.run_bass_kernel_spmd(
        nc, in_maps, core_ids=list(range(NCORES)), **kw)


def kernel(x, W, U, b):
    res = _run(_make_in_maps(x, W, U, b))
    return _unpack(res.results)


# revision 7
# speedup vs baseline: 2.0014x; 1.0038x over previous
"""Trainium2 Bass kernel for a 4-layer residual LSTM encoder (Keras-style).

Problem: x [64, 512, 512] fp32, per-layer kernels W/U [4, 512, 2048], bias
[4, 2048]; returns (rnn_output [64,512,512], states [4,64,512]) matching the
reference (states = layer-0 final hidden state replicated 4x).

Sharding: data-parallel, batch 64 -> 8 NeuronCores x 8. Each core runs the
full 4-layer LSTM on its batch shard.

Per layer the sequence is processed in segments of C timesteps with a
two-segment software pipeline inside an unrolled-2 For_i: while the
recurrence walks segment s, the input projection (zx = x@W + b) for segment
s+1 is computed by matmuls interleaved into the step stream (2 per step) and
kept entirely in SBUF. The recurrence streams hT through the PE with U
fp16-stationary (fast-weight-load eligible); gates are evaluated in a packed
[128, (gate, k, b)] layout on ACT/DVE so the new h lands directly in the
layout the next step's matmul needs (hidden on partitions, no transposes and
no DRAM roundtrips inside a layer).
"""

from contextlib import ExitStack

import numpy as np

import concourse.bass as bass
import concourse.mybir as mybir
import concourse.tile as tile
from concourse import bacc
from concourse import bass_utils

FP16 = mybir.dt.float16
FP32 = mybir.dt.float32
AF = mybir.ActivationFunctionType
ds = bass.ds
HINTS = tuple(mybir.ALL_ENGINES)

B, T, H, L = 64, 512, 512, 4
NCORES = 8
Bl = B // NCORES
C = 32                      # timesteps per segment
PAD = C * Bl                # input-column padding for the lookahead loads


def _build_lstm(nc, L=L, T=T, Bl=Bl, H=H, C=C):
    KT = H // 128            # 4 hidden chunks
    G4 = 4 * H               # 2048 gate columns
    TB = T * Bl              # activation free dim
    GW = KT * Bl             # 32: per-gate packed width (j4, b)
    PW = 2 * GW              # 64: gate-pair packed width (gi, j4, b)
    SW = C * Bl              # segment width in activation cols
    pad = C * Bl
    assert T % (2 * C) == 0

    xT = nc.dram_tensor("xT", [H, TB + pad], FP16, kind="ExternalInput").ap()
    W = nc.dram_tensor("W", [L, KT, 128, G4], FP16, kind="ExternalInput").ap()
    U = nc.dram_tensor("U", [L, KT, 128, G4], FP16, kind="ExternalInput").ap()
    bias = nc.dram_tensor("bias", [L, 128, 16], FP32, kind="ExternalInput").ap()
    yT = nc.dram_tensor("yT", [H, TB], FP16, kind="ExternalOutput").ap()
    h0f = nc.dram_tensor("h0f", [H, Bl], FP16, kind="ExternalOutput").ap()

    with tile.TileContext(nc) as tc:
        with ExitStack() as top:
            dram = top.enter_context(tc.tile_pool(name="dram", bufs=1, space="DRAM"))
            actA = dram.tile([H, TB + pad], FP16)
            actB = dram.tile([H, TB + pad], FP16)
            state = top.enter_context(tc.tile_pool(name="state", bufs=1))
            h_st = state.tile([128, GW], FP16)
            c_st = state.tile([128, GW], FP32)
            # zero the lookahead padding of the inter-layer buffers
            zpad = state.tile([128, pad], FP16)
            nc.vector.memset(zpad[:], 0.0)
            for buf in (actA, actB):
                for k in range(KT):
                    nc.sync.dma_start(
                        buf[k * 128:(k + 1) * 128, TB:TB + pad], zpad[:])

            for l in range(L):
                inp = xT if l == 0 else (actA if l % 2 == 1 else actB)
                outb = yT if l == L - 1 else (actA if l % 2 == 0 else actB)
                with ExitStack() as ls:
                    wp = ls.enter_context(tc.tile_pool(name=f"wp{l}", bufs=1))
                    U_sb, W_sb = [], []
                    for k in range(KT):
                        u_t = wp.tile([128, G4], FP16, name=f"u{l}_{k}")
                        nc.sync.dma_start(u_t[:], U[l, k])
                        U_sb.append(u_t)
                        w_t = wp.tile([128, G4], FP16, name=f"w{l}_{k}")
                        nc.sync.dma_start(w_t[:], W[l, k])
                        W_sb.append(w_t)
                    b_sb = wp.tile([128, 16], FP32, name=f"b{l}")
                    nc.sync.dma_start(b_sb[:], bias[l])

                    # persistent ping-pong buffers for the segment pipeline
                    pp = ls.enter_context(tc.tile_pool(name=f"pp{l}", bufs=1))
                    # in_pp[p][k]: input chunk (k-tile) [128, SW]
                    in_pp = [[pp.tile([128, SW], FP16, name=f"in{l}_{p}_{k}")
                              for k in range(KT)] for p in range(2)]
                    # zx_pp[p][pair]: projected input segment [128, C*PW]
                    # free layout (t, gi, j4, b); pair 0 = gates (i,f), 1 = (g,o)
                    zx_pp = [[pp.tile([128, C * PW], FP16, name=f"zx{l}_{p}_{d}")
                              for d in range(2)] for p in range(2)]

                    paps = ls.enter_context(
                        tc.tile_pool(name=f"paps{l}", bufs=2, space="PSUM"))
                    pbps = ls.enter_context(
                        tc.tile_pool(name=f"pbps{l}", bufs=2, space="PSUM"))
                    pbg = ls.enter_context(tc.tile_pool(name=f"pbg{l}", bufs=2))
                    pout = ls.enter_context(tc.tile_pool(name=f"pout{l}", bufs=2))

                    def in_chunk_dma(dst_tiles, col0):
                        for k in range(KT):
                            nc.sync.dma_start(
                                dst_tiles[k][:],
                                inp[k * 128:(k + 1) * 128, ds(col0, SW)])

                    def proj_mms(in_tiles, zx_tiles, mlist):
                        """Emit phase-A matmuls (index list) for one segment.

                        MM m: jt = m//KT, k = m%KT; after the last k, evacuate
                        psum (+bias, cast fp16) into zx_tiles with a strided
                        DVE write at (t, gi, j4, b) position j4 of pair/gi.
                        """
                        for m in mlist:
                            jt, k = m // KT, m % KT
                            if k == 0:
                                proj_mms.cur[jt] = paps.tile(
                                    [128, SW], FP32, tag="pa_ps", name=f"pa{l}")
                            nc.tensor.matmul(
                                proj_mms.cur[jt][:],
                                lhsT=W_sb[k][:, jt * 128:(jt + 1) * 128],
                                rhs=in_tiles[k][:],
                                start=(k == 0), stop=(k == KT - 1),
                            )
                            if k == KT - 1:
                                pair, gi, j4 = jt // 8, (jt % 8) // KT, jt % KT
                                dst = zx_tiles[pair][:].rearrange(
                                    "p (t gi j b) -> p t gi j b",
                                    gi=2, j=KT, b=Bl)[:, :, gi, j4, :]
                                nc.vector.tensor_scalar_add(
                                    dst,
                                    proj_mms.cur[jt][:].rearrange(
                                        "p (t b) -> p t b", b=Bl),
                                    b_sb[:, jt:jt + 1])
                    proj_mms.cur = [None] * 16

                    def segment(p, col0, zx_in_p, write_h0f, emit_proj,
                                in_res, in_next, zx_out_p):
                        """One segment of C recurrence steps (+ interleaved
                        projection matmuls for the next segment)."""
                        zxr = zx_pp[zx_in_p]
                        outc = [pout.tile([128, SW], FP16, tag=f"outc{k}",
                                          name=f"outc{l}_{k}")
                                for k in range(KT)]
                        nproj = (64 + C - 1) // C
                        for t in range(C):
                            ps_pair = [pbps.tile([128, PW], FP32, tag=f"psp{d}",
                                                 name=f"psp{l}_{d}")
                                       for d in range(2)]
                            zp = []
                            gtiles = []
                            for d in range(2):
                                for gi in range(2):
                                    for j4 in range(KT):
                                        jt = (d * 2 + gi) * KT + j4
                                        sl = slice((gi * KT + j4) * Bl,
                                                   (gi * KT + j4 + 1) * Bl)
                                        for k in range(KT):
                                            nc.tensor.matmul(
                                                ps_pair[d][:, sl],
                                                lhsT=U_sb[k][:, jt * 128:(jt + 1) * 128],
                                                rhs=h_st[:, k * Bl:(k + 1) * Bl],
                                                start=(k == 0), stop=(k == KT - 1),
                                            )
                                # z-add + activations for this pair right after
                                # its matmuls, so pair 0's chain overlaps
                                # pair 1's matmuls on the PE
                                zt = pbg.tile([128, PW], FP32, tag=f"z{d}",
                                              name=f"zt{l}_{d}")
                                nc.vector.tensor_add(
                                    zt[:], ps_pair[d][:],
                                    zxr[d][:, t * PW:(t + 1) * PW])
                                zp.append(zt)
                                if d == 0:
                                    g_if = pbg.tile([128, PW], FP32, tag="gif",
                                                    name=f"gif{l}")
                                    nc.scalar.activation(g_if[:], zt[:],
                                                         AF.Sigmoid)
                                else:
                                    g_go = pbg.tile([128, PW], FP32, tag="ggo",
                                                    name=f"ggo{l}")
                                    nc.scalar.activation(g_go[:, :GW],
                                                         zt[:, :GW], AF.Tanh)
                                    nc.scalar.activation(g_go[:, GW:],
                                                         zt[:, GW:], AF.Sigmoid)
                            if emit_proj:
                                proj_mms(in_next, zx_pp[zx_out_p],
                                         range(t * nproj, min(64, (t + 1) * nproj)))
                            t1 = pbg.tile([128, GW], FP32, tag="t1", name=f"t1{l}")
                            nc.vector.tensor_mul(t1[:], g_if[:, GW:], c_st[:])
                            t2 = pbg.tile([128, GW], FP32, tag="t2", name=f"t2{l}")
                            nc.vector.tensor_mul(t2[:], g_if[:, :GW], g_go[:, :GW])
                            nc.vector.tensor_add(c_st[:], t1[:], t2[:])
                            tc_t = pbg.tile([128, GW], FP32, tag="tc", name=f"tc{l}")
                            nc.scalar.activation(tc_t[:], c_st[:], AF.Tanh)
                            nc.vector.tensor_mul(h_st[:], g_go[:, GW:], tc_t[:])
                            for k in range(KT):
                                if l > 0:
                                    nc.vector.tensor_add(
                                        outc[k][:, t * Bl:(t + 1) * Bl],
                                        h_st[:, k * Bl:(k + 1) * Bl],
                                        in_res[k][:, t * Bl:(t + 1) * Bl])
                                else:
                                    nc.vector.tensor_copy(
                                        outc[k][:, t * Bl:(t + 1) * Bl],
                                        h_st[:, k * Bl:(k + 1) * Bl])
                        for k in range(KT):
                            nc.sync.dma_start(
                                outb[k * 128:(k + 1) * 128, ds(col0, SW)],
                                outc[k][:])
                        if write_h0f:
                            for k in range(KT):
                                nc.sync.dma_start(
                                    h0f[k * 128:(k + 1) * 128, :],
                                    h_st[:, k * Bl:(k + 1) * Bl])

                    # ---- prologue: chunk 0 + its projection; zero state ----
                    in_chunk_dma(in_pp[0], 0)
                    proj_mms(in_pp[0], zx_pp[0], range(64))
                    nc.vector.memset(h_st[:], 0.0)
                    nc.vector.memset(c_st[:], 0.0)

                    with tc.For_i(0, T, 2 * C, hint_engines=HINTS,
                                  name=f"rec{l}") as tv:
                        # body 0: segment at cols tv*Bl, using zx0/in0;
                        # loads chunk(s+1) -> in1, projects -> zx1
                        in_chunk_dma(in_pp[1], tv * Bl + SW)
                        segment(0, tv * Bl, 0, l == 0, True,
                                in_pp[0], in_pp[1], 1)
                        # body 1: segment at cols (tv+C)*Bl using zx1/in1;
                        # loads chunk(s+2) -> in0, projects -> zx0
                        in_chunk_dma(in_pp[0], tv * Bl + 2 * SW)
                        segment(1, tv * Bl + SW, 1, l == 0, True,
                                in_pp[1], in_pp[0], 0)
    return nc


_CACHE = {}


def _get_compiled():
    if "nc" not in _CACHE:
        nc = bacc.Bacc("TRN2", target_bir_lowering=False, debug=False,
                       num_devices=NCORES)
        _build_lstm(nc)
        nc.compile()
        _CACHE["nc"] = nc
    return _CACHE["nc"]


def _make_in_maps(x, W, U, b):
    KT = H // 128
    W4 = np.ascontiguousarray(
        np.asarray(W, np.float32).reshape(L, KT, 128, 4 * H)).astype(np.float16)
    U4 = np.ascontiguousarray(
        np.asarray(U, np.float32).reshape(L, KT, 128, 4 * H)).astype(np.float16)
    b4 = np.ascontiguousarray(
        np.asarray(b, np.float32).reshape(L, 16, 128).transpose(0, 2, 1))
    x = np.asarray(x, np.float32)
    in_maps = []
    for ci in range(NCORES):
        xs = x[ci * Bl:(ci + 1) * Bl]                       # [Bl, T, H]
        xTl = np.zeros((H, T * Bl + PAD), np.float16)
        xTl[:, :T * Bl] = xs.transpose(2, 1, 0).reshape(H, T * Bl)
        in_maps.append({"xT": xTl, "W": W4, "U": U4, "bias": b4})
    return in_maps


def _unpack(results):
    y = np.empty((B, T, H), np.float32)
    h0 = np.empty((B, H), np.float32)
    for ci in range(NCORES):
        yT = np.asarray(results[ci]["yT"], np.float32)       # [H, T*Bl]
        y[ci * Bl:(ci + 1) * Bl] = yT.reshape(H, T, Bl).transpose(2, 1, 0)
        h0[ci * Bl:(ci + 1) * Bl] = np.asarray(
            results[ci]["h0f"], np.float32).T                # [Bl, H]
    states = np.broadcast_to(h0, (L, B, H)).copy()
    return y, states


def _run(in_maps, **kw):
    nc = _get_compiled()
    return bass_utils.run_bass_kernel_spmd(
        nc, in_maps, core_ids=list(range(NCORES)), **kw)


def kernel(x, W, U, b):
    res = _run(_make_in_maps(x, W, U, b))
    return _unpack(res.results)
